# revision 1
# baseline (speedup 1.0000x reference)
"""Trainium2 Bass kernel for nn_GraphPatchEmbed (patch-embed conv + GCN layer).

Math: the whole module is linear in x.
  feats = patches(x) @ Wc.T            (2x2/stride-2 conv == per-patch matmul, K=12)
  xw    = feats @ gcn_w                -> xw = patches @ (Wc.T @ gcn_w) = P @ Wcomb
  out   = D^-1/2 (A+I') D^-1/2 xw + b  (graph aggregation; edges only touch batch 0,
                                        which is a 4-neighbor 256x256 grid stencil
                                        plus one extra edge (255,255)->(254,254))
Because aggregation acts on the node axis and the matmul on the channel axis, they
commute:  out = (D^-1/2 (A+I') D^-1/2 P) @ Wcomb + b.  The stencil is applied on the
host to the 12-row patch tensor (16x less data than the 192-channel features), the
bias is folded in as a 13th all-ones row of P / bias row of W, and the device kernel
is a single memory-bound matmul per core:
  [13, 32768] @ [13, 192] -> [32768, 192]   (8-way row-sharded over B*N = 262144)
"""

import numpy as np

from concourse import bacc, mybir, tile
import concourse.bass as bass
from concourse.bass_utils import run_bass_kernel_spmd

B, CIN, HIMG, WIMG = 4, 3, 512, 512
HG, WG = 256, 256          # grid after 2x2/stride-2 patching
N = HG * WG                # 65536 nodes per image
BN = B * N                 # 262144 total rows
EMB = 192
K = 13                     # 12 patch dims + 1 bias row
NCORES = 8
ROWS = BN // NCORES        # 32768 rows per core

_NC_CACHE = {}


GROUP = 8                      # matmul tiles per output DMA (6 KB runs/partition)
NT = 128                       # nodes per matmul tile (psum partition dim)
CH = 96                        # channels per c-chunk in the flipped kernel


def _build_nc_flip(nchunk=512, ogroup=4, psum_bufs=8, out_bufs=6, in_dt="bfloat16"):
    """W-stationary variant: out[c, node] c-major.

    The [node, c] kernel reloads its stationary (the q tile) into PE rows
    0-12 every matmul, so LDWEIGHTS|MATMUL serialize on the row-group
    conflict (~350 ns per 128 nodes, PE-bound at ~90 us). Here the
    stationary is a [13, 96] W chunk loaded twice in total; q streams as
    the moving operand (N=512 >= 4 us dense -> PE warms to 2.4 GHz).
    Host transposes the c-major output during unshard.
    """
    key = ("flip", nchunk, ogroup, psum_bufs, out_bufs, in_dt)
    if key in _NC_CACHE:
        return _NC_CACHE[key]
    nc = bacc.Bacc(
        "TRN2",
        target_bir_lowering=False,
        debug=False,
        enable_asserts=False,
        num_devices=NCORES,
    )
    f32 = mybir.dt.float32
    idt = getattr(mybir.dt, in_dt)
    q = nc.dram_tensor("q", [K, ROWS], idt, kind="ExternalInput").ap()
    w = nc.dram_tensor("w", [K, EMB], idt, kind="ExternalInput").ap()
    o = nc.dram_tensor("o", [EMB, ROWS], f32, kind="ExternalOutput").ap()

    OBLK = nchunk * ogroup          # nodes per output DMA (8 KB runs @ 2048)
    with tile.TileContext(nc) as tc:
        with (
            tc.tile_pool(name="wt", bufs=1) as wpool,
            tc.tile_pool(name="qp", bufs=1) as qpool,
            tc.tile_pool(name="ps", bufs=psum_bufs, space=bass.MemorySpace.PSUM) as pspool,
            tc.tile_pool(name="ot", bufs=out_bufs) as opool,
        ):
            wt = wpool.tile([K, EMB], idt)
            nc.sync.dma_start(out=wt[:], in_=w[:])
            # whole per-core q is 0.85 MB bf16 -> keep it SBUF-resident so the
            # two c-chunk passes both read it without a second HBM fetch
            qt = qpool.tile([K, ROWS], idt)
            NQD = 8
            for i in range(NQD):
                sl = slice(i * ROWS // NQD, (i + 1) * ROWS // NQD)
                nc.gpsimd.dma_start(out=qt[:, sl], in_=q[:, sl])
            t = 0
            for cc in range(EMB // CH):
                for g in range(ROWS // OBLK):
                    ot = opool.tile([CH, ogroup * nchunk], f32)
                    for j in range(ogroup):
                        n0 = g * OBLK + j * nchunk
                        ps = pspool.tile([CH, nchunk], f32)
                        nc.tensor.matmul(
                            ps[:], wt[:, cc * CH:(cc + 1) * CH],
                            qt[:, n0:n0 + nchunk],
                            start=True, stop=True,
                        )
                        if (t * ogroup + j) % 2 == 0:
                            nc.vector.tensor_copy(
                                ot[:, j * nchunk:(j + 1) * nchunk], ps[:])
                        else:
                            nc.scalar.copy(
                                ot[:, j * nchunk:(j + 1) * nchunk], ps[:])
                    eng = nc.sync if t % 2 == 0 else nc.gpsimd
                    eng.dma_start(
                        out=o[cc * CH:(cc + 1) * CH, g * OBLK:(g + 1) * OBLK],
                        in_=ot[:],
                    )
                    t += 1
    nc.compile()
    _NC_CACHE[key] = nc
    return nc


def _build_nc(chunk=4096, psum_bufs=8, out_bufs=10, q_bufs=4, in_dt="bfloat16"):
    key = (chunk, psum_bufs, out_bufs, q_bufs, in_dt)
    if key in _NC_CACHE:
        return _NC_CACHE[key]
    nc = bacc.Bacc(
        "TRN2",
        target_bir_lowering=False,
        debug=False,
        enable_asserts=False,
        num_devices=NCORES,
    )
    f32 = mybir.dt.float32
    # fp32 matmul costs 4 cycles/output-row (2 half-speed PE passes) and
    # disables fast weight load; bf16 is 1 cycle/row. PSUM accumulation
    # stays fp32 either way; inputs are O(1) and K=13, so bf16 input
    # rounding costs ~2e-3 relative error.
    idt = getattr(mybir.dt, in_dt)
    q = nc.dram_tensor("q", [K, ROWS], idt, kind="ExternalInput").ap()
    w = nc.dram_tensor("w", [K, EMB], idt, kind="ExternalInput").ap()
    o = nc.dram_tensor("o", [ROWS, EMB], f32, kind="ExternalOutput").ap()

    BLK = NT * GROUP           # 1024 nodes per output DMA
    with tile.TileContext(nc) as tc:
        with (
            tc.tile_pool(name="wt", bufs=1) as wpool,
            tc.tile_pool(name="qp", bufs=q_bufs) as qpool,
            tc.tile_pool(name="ps", bufs=psum_bufs, space=bass.MemorySpace.PSUM) as pspool,
            tc.tile_pool(name="ot", bufs=out_bufs) as opool,
        ):
            wt = wpool.tile([K, EMB], idt)
            nc.sync.dma_start(out=wt[:], in_=w[:])
            t = 0
            # ramped chunk schedule: the first output DMA (the roofline
            # resource) can only start after the first q chunk + 8 matmuls,
            # so keep the leading chunks small to cut the pipeline lead-in
            sched = [BLK, BLK, 2 * BLK, 4 * BLK]
            while sum(sched) + chunk <= ROWS:
                sched.append(chunk)
            sched[-1] += ROWS - sum(sched)
            off = 0
            for csz in sched:
                qt = qpool.tile([K, csz], idt)
                nc.sync.dma_start(out=qt[:], in_=q[:, off:off + csz])
                for g in range(csz // BLK):
                    # host permuted q columns so tile j / partition p computes
                    # node base + p*GROUP + j; partition p of the staging tile
                    # then holds GROUP consecutive output rows -> one DMA with
                    # GROUP*EMB*4 = 6 KB contiguous per partition
                    ot = opool.tile([NT, GROUP * EMB], f32)
                    base = g * BLK
                    for j in range(GROUP):
                        ps = pspool.tile([NT, EMB], f32)
                        nc.tensor.matmul(
                            ps[:], qt[:, base + j * NT: base + (j + 1) * NT], wt[:],
                            start=True, stop=True,
                        )
                        # split PSUM->SBUF copies ~5:4 DVE:ACT so neither
                        # engine serializes the 25 MB/core output stream
                        if (t * GROUP + j) % 9 < 5:
                            nc.vector.tensor_copy(ot[:, j * EMB:(j + 1) * EMB], ps[:])
                        else:
                            nc.scalar.copy(ot[:, j * EMB:(j + 1) * EMB], ps[:])
                    row0 = off + base
                    eng = nc.sync if t % 2 == 0 else nc.gpsimd
                    eng.dma_start(out=o[row0:row0 + BLK, :], in_=ot[:])
                    t += 1
                off += csz
    nc.compile()
    _NC_CACHE[key] = nc
    return nc


def _host_prep(x, conv_w, gcn_w, gcn_b):
    x = np.asarray(x, dtype=np.float32)
    conv_w = np.asarray(conv_w, dtype=np.float32)
    gcn_w = np.asarray(gcn_w, dtype=np.float32)
    gcn_b = np.asarray(gcn_b, dtype=np.float32)

    # patches P[b, k, n]: k = (cin, ki, kj), n = r*WG + c
    P = np.ascontiguousarray(
        x.reshape(B, CIN, HG, 2, WG, 2).transpose(0, 1, 3, 5, 2, 4)
    ).reshape(B, 12, N)

    # degrees with self-loops; grid edges exist only for batch 0
    nbr = np.full((HG, WG), 4.0, np.float32)
    nbr[0, :] -= 1; nbr[-1, :] -= 1; nbr[:, 0] -= 1; nbr[:, -1] -= 1
    deg = nbr + 1.0
    deg[HG - 2, WG - 2] += 1.0          # the module's trailing extra edge
    dr = (1.0 / np.sqrt(deg)).ravel()    # dinv per node

    # batch-0 aggregation applied to the patch rows (commutes with the matmul)
    z = (dr[None, :] * P[0]).reshape(12, HG, WG)
    s = z.copy()                          # self-loop term
    s[:, 1:, :] += z[:, :-1, :]
    s[:, :-1, :] += z[:, 1:, :]
    s[:, :, 1:] += z[:, :, :-1]
    s[:, :, :-1] += z[:, :, 1:]
    s[:, HG - 2, WG - 2] += z[:, HG - 1, WG - 1]
    Q0 = dr[None, :] * s.reshape(12, N)

    Q = np.empty((K, BN), np.float32)
    Q[:12, :N] = Q0
    Q[:12, N:] = P[1:].transpose(1, 0, 2).reshape(12, 3 * N)
    Q[12, :] = 1.0                        # bias row

    Wcomb = (conv_w.reshape(EMB, 12).astype(np.float64).T
             @ gcn_w.astype(np.float64)).astype(np.float32)
    Wfull = np.concatenate([Wcomb, gcn_b[None, :]], axis=0)  # (13, 192)
    return Q, Wfull


def kernel(x, conv_w, gcn_w, gcn_b, _trace=False, _nc_kwargs=None):
    Q, Wfull = _host_prep(x, conv_w, gcn_w, gcn_b)
    kw = dict(_nc_kwargs or {})
    nc = _build_nc(**kw)
    if kw.get("in_dt", "bfloat16") == "bfloat16":
        import ml_dtypes
        Q = Q.astype(ml_dtypes.bfloat16)
        Wfull = Wfull.astype(ml_dtypes.bfloat16)
    # permute columns within each 1024-node block: device tile j / partition p
    # reads column j*NT+p and must see node p*GROUP+j (see _build_nc)
    Qp = np.ascontiguousarray(
        Q.reshape(K, BN // (NT * GROUP), NT, GROUP).transpose(0, 1, 3, 2)
    ).reshape(K, BN)
    in_maps = [
        {"q": np.ascontiguousarray(Qp[:, c * ROWS:(c + 1) * ROWS]), "w": Wfull}
        for c in range(NCORES)
    ]
    res = run_bass_kernel_spmd(nc, in_maps, list(range(NCORES)), trace=_trace)
    out = np.concatenate([res.results[c]["o"] for c in range(NCORES)], axis=0)
    out = out.reshape(B, N, EMB)
    if _trace:
        return out, res
    return out



# revision 3
# speedup vs baseline: 1.2372x; 1.2372x over previous
"""Trainium2 Bass kernel for nn_GraphPatchEmbed (patch-embed conv + GCN layer).

Math: the whole module is linear in x.
  feats = patches(x) @ Wc.T            (2x2/stride-2 conv == per-patch matmul, K=12)
  xw    = feats @ gcn_w                -> xw = patches @ (Wc.T @ gcn_w) = P @ Wcomb
  out   = D^-1/2 (A+I') D^-1/2 xw + b  (graph aggregation; edges only touch batch 0)
Aggregation (node axis) and matmul (channel axis) commute, so the stencil is applied
on the host to the 12-row patch tensor, the bias folds in as a 13th all-ones row,
and the device kernel is one memory-bound matmul per core:
  [13, 32768] @ [13, 192] -> [192, 32768] c-major   (8-way row-sharded over B*N)

Device design (v3):
  - W-stationary: stationary = [13, 96] W chunk, q streams as moving operand
    (512 cols / ~213 ns per matmul; PE floor = 2 passes x 32768 cols = 27.3 us).
  - Output precision is the roofline: fp32 would be 25.2 MB/core (70 us at the
    358 GB/s HBM-per-NC limit). fp8e3 (E3M4, x4 pre-scale folded into W to dodge
    subnormals; host decodes and rescales) costs 1.33e-2 rel err against the
    2e-2 gate and cuts the stream to 6.3 MB/core (~22 us) — under the PE floor.
  - Output staging spans partitions 0-95 (SDMA ports 0-11); the q load would
    hotspot 2 ports if placed at one base, so its columns are split across all
    three legal matmul bases {0, 32, 64} (W replicated per base), putting
    ~0.28 MB on each of 6 ports instead of 0.85 MB on 2.
  - Channel chunks interleave per node-super-chunk so DMA demand stays level.
"""

import numpy as np

from concourse import bacc, mybir, tile
import concourse.bass as bass
from concourse.bass_utils import run_bass_kernel_spmd

B, CIN, HIMG, WIMG = 4, 3, 512, 512
HG, WG = 256, 256          # grid after 2x2/stride-2 patching
N = HG * WG                # 65536 nodes per image
BN = B * N                 # 262144 total rows
EMB = 192
K = 13                     # 12 patch dims + 1 bias row
NCORES = 8
ROWS = BN // NCORES        # 32768 rows per core
CH = 96                    # channels per c-chunk (psum partition dim)
NT = 512                   # nodes per matmul (one 2KB psum bank)
FP8_SCALE = 4.0            # folded into W before the e3m4 downcast

_NC_CACHE = {}

# q column ranges per stationary base {0, 32, 64}; ramped leading supers cut
# the pipeline lead-in. Sum of each base's supers = that base's column count.
BASE_SCHED = [
    (0,  [1024, 1024, 2048, 4096, 2560]),    # 10752 cols
    (32, [4096, 4096, 3072]),                # 11264 cols
    (64, [4096, 4096, 2560]),                # 10752 cols
]
assert sum(sum(s) for _, s in BASE_SCHED) == ROWS


def _build_nc(out_dts=("float8e3", "float8e3"), psum_bufs=8, out_bufs=3,
              dve_ratio=(1, 2)):
    key = (out_dts, psum_bufs, out_bufs, dve_ratio)
    if key in _NC_CACHE:
        return _NC_CACHE[key]
    nc = bacc.Bacc(
        "TRN2",
        target_bir_lowering=False,
        debug=False,
        enable_asserts=False,
        num_devices=NCORES,
    )
    f16 = mybir.dt.float16
    odt = [getattr(mybir.dt, d) for d in out_dts]
    q = nc.dram_tensor("q", [K, ROWS], f16, kind="ExternalInput").ap()
    w = nc.dram_tensor("w", [K, EMB], f16, kind="ExternalInput").ap()
    outs = [
        nc.dram_tensor(f"o{cc}", [CH, ROWS], odt[cc], kind="ExternalOutput").ap()
        for cc in range(2)
    ]

    with tile.TileContext(nc) as tc:
        with (
            tc.tile_pool(name="wt", bufs=1) as wpool,
            tc.tile_pool(name="qp", bufs=1) as qpool,
            tc.tile_pool(name="ps", bufs=psum_bufs, space=bass.MemorySpace.PSUM) as pspool,
            tc.tile_pool(name="o0", bufs=out_bufs) as opool0,
            tc.tile_pool(name="o1", bufs=out_bufs) as opool1,
        ):
            wt = wpool.tile([128, EMB], f16)
            QCOLS = max(sum(s) for _, s in BASE_SCHED)
            qt = qpool.tile([128, QCOLS], f16)
            # replicate W at each stationary base; q columns split across the
            # bases so the input DMA spreads over 6 SDMA ports, interleaved
            # with the compute order so each chunk lands just ahead of use
            for base, _ in BASE_SCHED:
                nc.scalar.dma_start(out=wt[base:base + K, :], in_=w[:])
            plan = []   # (base, global col0, local col0, csz)
            goff = 0
            for base, sched in BASE_SCHED:
                loff = 0
                for csz in sched:
                    plan.append((base, goff, loff, csz))
                    goff += csz
                    loff += csz
            for base, gc0, lc0, csz in plan:
                nc.scalar.dma_start(out=qt[base:base + K, lc0:lc0 + csz],
                                    in_=q[:, gc0:gc0 + csz])

            opools = [opool0, opool1]
            t = 0   # output-DMA index
            v = 0   # copy index (DVE/ACT split)
            for base, gc0, lc0, csz in plan:
                for cc in range(2):
                    ot = opools[cc].tile([CH, csz], odt[cc])
                    for j in range(csz // NT):
                        ps = pspool.tile([CH, NT], mybir.dt.float32)
                        lc = lc0 + j * NT
                        nc.tensor.matmul(
                            ps[:],
                            wt[base:base + K, cc * CH:(cc + 1) * CH],
                            qt[base:base + K, lc:lc + NT],
                            start=True, stop=True,
                        )
                        # split psum->sbuf cast copies across DVE and ACT
                        if v % dve_ratio[1] < dve_ratio[0]:
                            nc.vector.tensor_copy(ot[:, j * NT:(j + 1) * NT], ps[:])
                        else:
                            nc.scalar.copy(ot[:, j * NT:(j + 1) * NT], ps[:])
                        v += 1
                    eng = nc.sync if t % 2 == 0 else nc.gpsimd
                    eng.dma_start(out=outs[cc][:, gc0:gc0 + csz], in_=ot[:])
                    t += 1
    nc.compile()
    _NC_CACHE[key] = nc
    return nc


def _host_prep(x, conv_w, gcn_w, gcn_b):
    x = np.asarray(x, dtype=np.float32)
    conv_w = np.asarray(conv_w, dtype=np.float32)
    gcn_w = np.asarray(gcn_w, dtype=np.float32)
    gcn_b = np.asarray(gcn_b, dtype=np.float32)

    # patches P[b, k, n]: k = (cin, ki, kj), n = r*WG + c
    P = np.ascontiguousarray(
        x.reshape(B, CIN, HG, 2, WG, 2).transpose(0, 1, 3, 5, 2, 4)
    ).reshape(B, 12, N)

    # degrees with self-loops; grid edges exist only for batch 0
    nbr = np.full((HG, WG), 4.0, np.float32)
    nbr[0, :] -= 1; nbr[-1, :] -= 1; nbr[:, 0] -= 1; nbr[:, -1] -= 1
    deg = nbr + 1.0
    deg[HG - 2, WG - 2] += 1.0          # the module's trailing extra edge
    dr = (1.0 / np.sqrt(deg)).ravel()    # dinv per node

    # batch-0 aggregation applied to the patch rows (commutes with the matmul)
    z = (dr[None, :] * P[0]).reshape(12, HG, WG)
    s = z.copy()                          # self-loop term
    s[:, 1:, :] += z[:, :-1, :]
    s[:, :-1, :] += z[:, 1:, :]
    s[:, :, 1:] += z[:, :, :-1]
    s[:, :, :-1] += z[:, :, 1:]
    s[:, HG - 2, WG - 2] += z[:, HG - 1, WG - 1]
    Q0 = dr[None, :] * s.reshape(12, N)

    Q = np.empty((K, BN), np.float32)
    Q[:12, :N] = Q0
    Q[:12, N:] = P[1:].transpose(1, 0, 2).reshape(12, 3 * N)
    Q[12, :] = 1.0                        # bias row

    Wcomb = (conv_w.reshape(EMB, 12).astype(np.float64).T
             @ gcn_w.astype(np.float64)).astype(np.float32)
    Wfull = np.concatenate([Wcomb, gcn_b[None, :]], axis=0)  # (13, 192)
    return Q, Wfull


def kernel(x, conv_w, gcn_w, gcn_b, _trace=False, _nc_kwargs=None):
    Q, Wfull = _host_prep(x, conv_w, gcn_w, gcn_b)
    kw = dict(_nc_kwargs or {})
    nc = _build_nc(**kw)
    out_dts = kw.get("out_dts", ("float8e3", "float8e3"))
    Wdev = Wfull.copy()
    for cc in range(2):
        if out_dts[cc] == "float8e3":
            Wdev[:, cc * CH:(cc + 1) * CH] *= FP8_SCALE
    Q16 = Q.astype(np.float16)
    W16 = Wdev.astype(np.float16)
    in_maps = [
        {"q": np.ascontiguousarray(Q16[:, c * ROWS:(c + 1) * ROWS]), "w": W16}
        for c in range(NCORES)
    ]
    res = run_bass_kernel_spmd(nc, in_maps, list(range(NCORES)), trace=_trace)
    out = np.empty((BN, EMB), np.float32)
    for c in range(NCORES):
        r = res.results[c]
        sl = slice(c * ROWS, (c + 1) * ROWS)
        for cc in range(2):
            o = r[f"o{cc}"].astype(np.float32).T
            if out_dts[cc] == "float8e3":
                o *= (1.0 / FP8_SCALE)
            out[sl, cc * CH:(cc + 1) * CH] = o
    out = out.reshape(B, N, EMB)
    if _trace:
        return out, res
    return out
